# revision 1
# baseline (speedup 1.0000x reference)
"""BlockGRUCell Trainium2 kernel.

Computation (per reference):
  hx = concat([h, x], -1)                       # (B, 2048)
  gate[b, 192g+o] = sum_i hx[b, 128g+i] * W[g, o, i]   # block-diagonal matmul
  r, c, u = split(gate + bias, 3)               # bias == 0 from setup_inputs
  h_new = sigmoid(u) * tanh(sigmoid(r) * c) + (1 - sigmoid(u)) * h

Sharding: data-parallel over batch across 8 NeuronCores (2048 rows each),
weights replicated.

The TensorE matmul contracts over the partition dim, so the stationary
operand must be hx^T per 128-feature block. The host pre-packs hx into
per-tile transposed bf16 panels (doing this on device costs a PE transpose
plus a PSUM->SBUF cast that saturates VectorE/ScalarE):
  hxt[t, p, 128g+b] = hx[128t+b, 128g+p]

Per core, per 128-row tile:
  - DMA: hxt tile (bf16 transposed panel, 512K), h tile (fp32, 512K)
  - 20 block matmuls (bf16, fp32 accum) into three [128, 1024] PSUM panels
    (= r/c/u exactly; matmuls split at PSUM bank crossings); pool bufs=4
    so the next tile's r-matmuls start as soon as one panel frees
  - ScalarE: sigmoid(r), tanh(reset*c), sigmoid(u)
  - VectorE: rc from PSUM and the blend h + upd*(cand - h); fp32
    tensor_tensor is 1x everywhere and GpSimd would steal DVE's second
    read port, so everything elementwise stays on VectorE
"""

import numpy as np
import ml_dtypes

import concourse.bass as bass
import concourse.bacc as bacc
import concourse.tile as tile
import concourse.mybir as mybir
from concourse.bass_utils import run_bass_kernel_spmd

N_CORES = 8
BATCH = 16384
BS = BATCH // N_CORES            # rows per core
P = 128
NT = BS // P                     # 128-row tiles per core
HID = 1024
G = 16                           # feature blocks
IN_PER = 128
OUT_PER = 192
GATE = 3 * HID                   # 3072
PSUM_BANK_F32 = 512

F32 = mybir.dt.float32
BF16 = mybir.dt.bfloat16
F16 = mybir.dt.float16
AFT = mybir.ActivationFunctionType


def _body(tc, nc, hxt_d, h_d, wt_d, out_d):
    with (
        tc.tile_pool(name="consts", bufs=1) as consts,
        tc.tile_pool(name="io", bufs=6) as io,
        tc.tile_pool(name="panels", bufs=4) as panels,
        tc.tile_pool(name="gatep", bufs=4, space="PSUM") as gatep,
    ):
        # warm the sigmoid/tanh ACT table during the initial DMAs (the
        # ~2.7us ACT_TABLE_LOAD otherwise lands on tile 0's critical path)
        warm = consts.tile([P, 1], F32)
        nc.vector.memset(warm, 0.0)
        nc.scalar.activation(warm, warm, AFT.Sigmoid)

        # split the weight load so tile 0's r-gate matmuls start sooner
        wt_s = consts.tile([P, G * OUT_PER], BF16)
        nc.sync.dma_start(out=wt_s[:, 0:GATE // 2], in_=wt_d[:, 0:GATE // 2])
        nc.sync.dma_start(out=wt_s[:, GATE // 2:], in_=wt_d[:, GATE // 2:])

        h2 = None
        out2 = None
        for t in range(NT):
            hxt = io.tile([P, G * P], BF16, tag="hxt")
            if t == 0:
                nc.sync.dma_start(out=hxt[:, 0:G * P // 2],
                                  in_=hxt_d[0, :, 0:G * P // 2])
                nc.sync.dma_start(out=hxt[:, G * P // 2:],
                                  in_=hxt_d[0, :, G * P // 2:])
            else:
                nc.sync.dma_start(out=hxt, in_=hxt_d[t])
            if t % 2 == 0:
                # h arrives pair-packed: one 1 MiB DMA per two tiles.
                # For the first pair, defer the load until after the matmul
                # feeds so it doesn't compete with the critical-path DMAs.
                h2 = io.tile([P, 2 * HID], F32, tag="h2", bufs=4)
                if t > 0:
                    nc.sync.dma_start(out=h2, in_=h_d[t // 2])
                out2 = io.tile([P, 2 * HID], F32, tag="out2", bufs=4)
            h_t = h2[:, (t % 2) * HID:(t % 2 + 1) * HID]

            # gate panels = the r/c/u split exactly (2 PSUM banks each)
            gR = gatep.tile([P, HID], F32, tag="gate")
            gC = gatep.tile([P, HID], F32, tag="gate")
            gU = gatep.tile([P, HID], F32, tag="gate")
            gs = (gR, gC, gU)

            for g in range(G):
                lhsT = hxt[:, g * P:(g + 1) * P]
                w0 = g * OUT_PER
                # split matmul writes at PSUM bank (512) boundaries
                c0 = w0
                while c0 < w0 + OUT_PER:
                    c1 = min(w0 + OUT_PER,
                             (c0 // PSUM_BANK_F32 + 1) * PSUM_BANK_F32)
                    gate = gs[c0 // HID]
                    nc.tensor.matmul(gate[:, c0 % HID:(c0 % HID) + c1 - c0],
                                     lhsT, wt_s[:, c0:c1],
                                     start=True, stop=True)
                    c0 = c1

            if t == 0:
                nc.sync.dma_start(out=h2, in_=h_d[0])

            reset = panels.tile([P, HID], F32, tag="reset")
            rc = panels.tile([P, HID], F32, tag="rc")
            cand = panels.tile([P, HID], F32, tag="cand")
            upd = panels.tile([P, HID], F32, tag="upd")
            dd = panels.tile([P, HID], F32, tag="dd")
            ee = panels.tile([P, HID], F32, tag="ee")
            hn = out2[:, (t % 2) * HID:(t % 2 + 1) * HID]

            # the last tile's epilogue runs in column halves so its serial
            # ACT<->DVE chain (fully exposed at the end of the pipeline)
            # drains finer-grained, and the final store streams out early
            splits = [(0, HID)] if t < NT - 1 else \
                     [(0, HID // 2), (HID // 2, HID)]
            for idx, (a, b) in enumerate(splits):
                nc.scalar.activation(reset[:, a:b], gR[:, a:b], AFT.Sigmoid)
                nc.vector.tensor_tensor(rc[:, a:b], gC[:, a:b],
                                        reset[:, a:b], mybir.AluOpType.mult)
                nc.scalar.activation(cand[:, a:b], rc[:, a:b], AFT.Tanh)
                nc.scalar.activation(upd[:, a:b], gU[:, a:b], AFT.Sigmoid)
                # h_new = h + upd*(cand - h)
                nc.vector.tensor_sub(dd[:, a:b], cand[:, a:b], h_t[:, a:b])
                nc.vector.tensor_mul(ee[:, a:b], upd[:, a:b], dd[:, a:b])
                nc.vector.tensor_add(hn[:, a:b], h_t[:, a:b], ee[:, a:b])
                if t == NT - 1:
                    lo = 0 if idx == 0 else HID + a
                    nc.sync.dma_start(out=out_d[t // 2][:, lo:HID + b],
                                      in_=out2[:, lo:HID + b])
            if t % 2 == 1 and t != NT - 1:
                nc.sync.dma_start(out=out_d[t // 2], in_=out2)


_NC_CACHE = {}


def _build_nc():
    if "nc" in _NC_CACHE:
        return _NC_CACHE["nc"]
    nc = bacc.Bacc()
    hxt_d = nc.dram_tensor("hxt", [NT, P, G * P], BF16, kind="ExternalInput")
    h_d = nc.dram_tensor("h2", [NT // 2, P, 2 * HID], F32,
                         kind="ExternalInput")
    wt_d = nc.dram_tensor("wt", [P, G * OUT_PER], BF16, kind="ExternalInput")
    out_d = nc.dram_tensor("out", [NT // 2, P, 2 * HID], F32,
                           kind="ExternalOutput")
    with tile.TileContext(nc) as tc:
        _body(tc, nc, hxt_d, h_d, wt_d, out_d)
    nc.compile()
    _NC_CACHE["nc"] = nc
    return nc


def _np_reference(x, h, weight, bias):
    hx = np.concatenate([h, x], axis=-1)
    xg = hx.reshape(x.shape[0], G, IN_PER)
    gate = np.einsum("bgi,goi->bgo", xg, weight).reshape(x.shape[0], GATE)
    gate = gate + bias
    r, c, u = np.split(gate, 3, axis=-1)
    reset = 1.0 / (1.0 + np.exp(-r))
    cand = np.tanh(reset * c)
    upd = 1.0 / (1.0 + np.exp(-u))
    return (upd * cand + (1.0 - upd) * h).astype(np.float32)


def _pack_hxt(hs, xs):
    """-> [NT, 128, 2048] bf16 with hxt[t, p, 128g+b] = hx[128t+b, 128g+p],
    where hx = concat([h, x], -1) per-row (blocks 0-7 = h, 8-15 = x)."""
    def tp(a):                      # [BS, 1024] -> [NT, 128, 8, 128]
        return a.reshape(NT, P, 8, P).transpose(0, 3, 2, 1)   # [t, p, g, b]
    arr = np.concatenate([tp(hs), tp(xs)], axis=2)            # [t, p, 16, b]
    return np.ascontiguousarray(arr.reshape(NT, P, G * P)).astype(
        ml_dtypes.bfloat16)


def _pack_pairs(a):
    """[BS, 1024] -> [NT//2, 128, 2048] with [q, p, 1024s+f] = a[256q+128s+p, f]."""
    return np.ascontiguousarray(
        a.reshape(NT // 2, 2, P, HID).transpose(0, 2, 1, 3)
        .reshape(NT // 2, P, 2 * HID))


def _unpack_pairs(a):
    """inverse of _pack_pairs."""
    return np.ascontiguousarray(
        a.reshape(NT // 2, P, 2, HID).transpose(0, 2, 1, 3).reshape(BS, HID))


def _run(x, h, weight, bias, trace=False, tmpdir=None):
    # wt[p, 192g+o] = W[g, o, p] — the exact SBUF layout, one contiguous DMA
    wt = np.ascontiguousarray(
        weight.transpose(2, 0, 1).reshape(P, G * OUT_PER)).astype(
        ml_dtypes.bfloat16)
    nc = _build_nc()
    in_maps = []
    for c in range(N_CORES):
        sl = slice(c * BS, (c + 1) * BS)
        xs, hs = x[sl], h[sl]
        in_maps.append({
            "hxt": _pack_hxt(hs, xs),
            "h2": _pack_pairs(hs),
            "wt": wt,
        })
    res = run_bass_kernel_spmd(nc, in_maps, core_ids=list(range(N_CORES)),
                               trace=trace, tmpdir=tmpdir)
    out = np.concatenate([_unpack_pairs(m["out"]) for m in res.results],
                         axis=0)
    return out, res


def kernel(x, h, weight, bias):
    x = np.asarray(x, dtype=np.float32)
    h = np.asarray(h, dtype=np.float32)
    weight = np.asarray(weight, dtype=np.float32)
    bias = np.asarray(bias, dtype=np.float32)
    if np.any(bias != 0.0):
        # setup_inputs() always passes zero bias; keep a correct fallback.
        return _np_reference(x, h, weight, bias)
    out, _ = _run(x, h, weight, bias)
    return out



# revision 2
# speedup vs baseline: 1.3062x; 1.3062x over previous
"""BlockGRUCell Trainium2 kernel.

Computation (per reference):
  hx = concat([h, x], -1)                       # (B, 2048)
  gate[b, 192g+o] = sum_i hx[b, 128g+i] * W[g, o, i]   # block-diagonal matmul
  r, c, u = split(gate + bias, 3)               # bias == 0 from setup_inputs
  h_new = sigmoid(u) * tanh(sigmoid(r) * c) + (1 - sigmoid(u)) * h

Sharding: data-parallel over batch across 8 NeuronCores (2048 rows each),
weights replicated.

v2 design (from trace analysis of the 95.7us baseline):
  - The baseline was DVE-bound (fp32 tensor_tensor is 1x ~= 1152ns per
    1024-wide op, 78us busy) with ACT (55us) and DMA (24MB -> 72us) close
    behind.  Fixes:
  - All elementwise tensors are bf16 -> DVE tensor_tensor runs 2x_1P
    (2 elem/cycle/lane).  Only the rc = sigmoid(r)*c multiply reads PSUM
    (f32, 1x) since TRN2 matmul can only write f32 PSUM.
  - h is loaded as bf16 (not f32) and the output is stored as bf16 and
    upcast on host: HBM traffic 24MB -> 16MB per core.
  - Tiles processed in pairs of 128 rows: 1MB hxt loads, 512KB h/out
    transfers, and the tanh + blend run at free-dim 2048 to amortize the
    per-op overhead (and halve semaphore count).
  - Output stores go through the SWDGE (gpsimd) DMA ring so they never
    block the HWDGE (sync) load ring's FIFO dispatch.

Per core, per 128-row tile: 20 block matmuls (bf16, f32 PSUM, split at
512-col PSUM bank crossings) into gR/gC/gU [128,1024] panels; ACT does
sigmoid(gR)->reset, sigmoid(gU)->upd (bf16, into pair tiles); DVE does
rc = gC*reset (1x, PSUM).  Per pair: ACT tanh(rc_pair)->cand (2048 wide);
DVE: d = cand - h, e = upd*d (per-tile halves), hn = h + e; store.
"""

import numpy as np
import ml_dtypes

import concourse.bass as bass
import concourse.bacc as bacc
import concourse.tile as tile
import concourse.mybir as mybir
from concourse.bass_utils import run_bass_kernel_spmd

N_CORES = 8
BATCH = 16384
BS = BATCH // N_CORES            # rows per core
P = 128
NT = BS // P                     # 128-row tiles per core
NP = NT // 2                     # tile pairs per core
HID = 1024
G = 16                           # feature blocks
IN_PER = 128
OUT_PER = 192
GATE = 3 * HID                   # 3072
PSUM_BANK_F32 = 512

F32 = mybir.dt.float32
BF16 = mybir.dt.bfloat16
AFT = mybir.ActivationFunctionType
MULT = mybir.AluOpType.mult

# dtype knobs (bf16 is the accuracy-safe default; fp8e4 hxt is a possible
# future DMA/LDWEIGHTS optimization with ~1.6x error margin left)
HXT_DT = BF16
WT_DT = BF16
WS = 1.0                         # weight pre-scale, unwound via ACT scale


def _body(tc, nc, hxt_d, h_d, wt_d, out_d):
    inv_ws = 1.0 / WS
    with (
        tc.tile_pool(name="consts", bufs=1) as consts,
        tc.tile_pool(name="io", bufs=3) as io,
        tc.tile_pool(name="panels", bufs=2) as panels,
        tc.tile_pool(name="gatep", bufs=4, space="PSUM") as gatep,
    ):
        # warm the sigmoid/tanh ACT table during the initial DMAs (the
        # ~2.7us ACT_TABLE_LOAD otherwise lands on tile 0's critical path)
        warm = consts.tile([P, 1], F32)
        nc.vector.memset(warm, 0.0)
        nc.scalar.activation(warm, warm, AFT.Sigmoid)

        # split the weight load so tile 0's r-gate matmuls start sooner
        wt_s = consts.tile([P, GATE], WT_DT)
        nc.sync.dma_start(out=wt_s[:, 0:GATE // 2], in_=wt_d[:, 0:GATE // 2])
        nc.sync.dma_start(out=wt_s[:, GATE // 2:], in_=wt_d[:, GATE // 2:])

        for q in range(NP):
            hxt = io.tile([P, 2 * G * P], HXT_DT, tag="hxt")
            if q == 0:
                nc.sync.dma_start(out=hxt[:, 0:G * P], in_=hxt_d[0, :, 0:G * P])
                nc.sync.dma_start(out=hxt[:, G * P:], in_=hxt_d[0, :, G * P:])
            else:
                nc.sync.dma_start(out=hxt, in_=hxt_d[q])
            h2 = io.tile([P, 2 * HID], BF16, tag="h2")
            if q > 0:
                nc.sync.dma_start(out=h2, in_=h_d[q])
            out2 = io.tile([P, 2 * HID], BF16, tag="out2")

            reset2 = panels.tile([P, 2 * HID], BF16, tag="reset")
            upd2 = panels.tile([P, 2 * HID], BF16, tag="upd")
            rc2 = panels.tile([P, 2 * HID], BF16, tag="rc")
            cand2 = panels.tile([P, 2 * HID], BF16, tag="cand")
            dd = panels.tile([P, 2 * HID], BF16, tag="dd")
            ee = panels.tile([P, 2 * HID], BF16, tag="ee")

            for s in (0, 1):
                gR = gatep.tile([P, HID], F32, tag="gate")
                gC = gatep.tile([P, HID], F32, tag="gate")
                gU = gatep.tile([P, HID], F32, tag="gate")
                gs = (gR, gC, gU)

                for g in range(G):
                    lhsT = hxt[:, 2 * HID * s + g * P:2 * HID * s + (g + 1) * P]
                    w0 = g * OUT_PER
                    # split matmul writes at PSUM bank (512 f32) boundaries
                    c0 = w0
                    while c0 < w0 + OUT_PER:
                        c1 = min(w0 + OUT_PER,
                                 (c0 // PSUM_BANK_F32 + 1) * PSUM_BANK_F32)
                        gate = gs[c0 // HID]
                        nc.tensor.matmul(
                            gate[:, c0 % HID:(c0 % HID) + c1 - c0],
                            lhsT, wt_s[:, c0:c1], start=True, stop=True)
                        c0 = c1

                if q == 0 and s == 0:
                    # defer the first h load until after the critical-path
                    # hxt/wt DMAs are queued (h is only needed at the blend)
                    nc.sync.dma_start(out=h2, in_=h_d[0])

                sl = slice(s * HID, (s + 1) * HID)
                nc.scalar.activation(reset2[:, sl], gR, AFT.Sigmoid,
                                     scale=inv_ws)
                nc.scalar.activation(upd2[:, sl], gU, AFT.Sigmoid,
                                     scale=inv_ws)
                nc.vector.tensor_tensor(rc2[:, sl], gC, reset2[:, sl], MULT)

            # pair epilogue at free-dim 2048; the last pair runs in halves
            # so its serial ACT<->DVE tail drains finer-grained and the
            # final store streams out early
            splits = [(0, 2 * HID)] if q < NP - 1 else \
                     [(0, HID), (HID, 2 * HID)]
            for a, b in splits:
                nc.scalar.activation(cand2[:, a:b], rc2[:, a:b], AFT.Tanh,
                                     scale=inv_ws)
                nc.vector.tensor_sub(dd[:, a:b], cand2[:, a:b], h2[:, a:b])
                # e = upd * d in per-tile halves (upd lives in pair halves)
                m = (a + b) // 2
                if b - a == 2 * HID:
                    nc.vector.tensor_mul(ee[:, a:m], upd2[:, a:m], dd[:, a:m])
                    nc.vector.tensor_mul(ee[:, m:b], upd2[:, m:b], dd[:, m:b])
                else:
                    nc.vector.tensor_mul(ee[:, a:b], upd2[:, a:b], dd[:, a:b])
                nc.vector.tensor_add(out2[:, a:b], h2[:, a:b], ee[:, a:b])
                # stores ride the SWDGE (gpsimd) ring: the HWDGE load ring
                # dispatches FIFO, so a store waiting on compute would stall
                # the next pair's loads
                nc.gpsimd.dma_start(out=out_d[q][:, a:b], in_=out2[:, a:b])


_NC_CACHE = {}


def _build_nc():
    if "nc" in _NC_CACHE:
        return _NC_CACHE["nc"]
    nc = bacc.Bacc()
    hxt_d = nc.dram_tensor("hxt", [NP, P, 2 * G * P], HXT_DT,
                           kind="ExternalInput")
    h_d = nc.dram_tensor("h2", [NP, P, 2 * HID], BF16, kind="ExternalInput")
    wt_d = nc.dram_tensor("wt", [P, GATE], WT_DT, kind="ExternalInput")
    out_d = nc.dram_tensor("out", [NP, P, 2 * HID], BF16,
                           kind="ExternalOutput")
    with tile.TileContext(nc) as tc:
        _body(tc, nc, hxt_d, h_d, wt_d, out_d)
    nc.compile()
    _NC_CACHE["nc"] = nc
    return nc


def _np_reference(x, h, weight, bias):
    hx = np.concatenate([h, x], axis=-1)
    xg = hx.reshape(x.shape[0], G, IN_PER)
    gate = np.einsum("bgi,goi->bgo", xg, weight).reshape(x.shape[0], GATE)
    gate = gate + bias
    r, c, u = np.split(gate, 3, axis=-1)
    reset = 1.0 / (1.0 + np.exp(-r))
    cand = np.tanh(reset * c)
    upd = 1.0 / (1.0 + np.exp(-u))
    return (upd * cand + (1.0 - upd) * h).astype(np.float32)


def _np_dt(dt):
    return {BF16: ml_dtypes.bfloat16,
            mybir.dt.float8e4: ml_dtypes.float8_e4m3}[dt]


def _pack_hxt(hs, xs):
    """-> [NP, 128, 4096] with hxt[q, p, 2048s+128g+b] = hx[256q+128s+b,
    128g+p], where hx = concat([h, x], -1) (blocks 0-7 = h, 8-15 = x)."""
    def tp(a):                      # [BS, 1024] -> [NT, 128, 8, 128]
        return a.reshape(NT, P, 8, P).transpose(0, 3, 2, 1)   # [t, p, g, b]
    arr = np.concatenate([tp(hs), tp(xs)], axis=2)            # [t, p, 16, b]
    arr = arr.reshape(NP, 2, P, 2 * HID).transpose(0, 2, 1, 3)
    return np.ascontiguousarray(arr.reshape(NP, P, 4 * HID)).astype(
        _np_dt(HXT_DT))


def _pack_pairs(a):
    """[BS, 1024] -> [NP, 128, 2048] with [q, p, 1024s+f] = a[256q+128s+p, f]."""
    return np.ascontiguousarray(
        a.reshape(NP, 2, P, HID).transpose(0, 2, 1, 3)
        .reshape(NP, P, 2 * HID))


def _unpack_pairs(a):
    """inverse of _pack_pairs."""
    return np.ascontiguousarray(
        a.reshape(NP, P, 2, HID).transpose(0, 2, 1, 3).reshape(BS, HID))


def _run(x, h, weight, bias, trace=False, tmpdir=None):
    # wt[p, 192g+o] = W[g, o, p] — the exact SBUF layout, one contiguous DMA
    wt = np.ascontiguousarray(
        (weight * WS).transpose(2, 0, 1).reshape(P, GATE)).astype(
        _np_dt(WT_DT))
    nc = _build_nc()
    in_maps = []
    for c in range(N_CORES):
        sl = slice(c * BS, (c + 1) * BS)
        xs, hs = x[sl], h[sl]
        in_maps.append({
            "hxt": _pack_hxt(hs, xs),
            "h2": _pack_pairs(hs).astype(ml_dtypes.bfloat16),
            "wt": wt,
        })
    res = run_bass_kernel_spmd(nc, in_maps, core_ids=list(range(N_CORES)),
                               trace=trace, tmpdir=tmpdir)
    out = np.concatenate(
        [_unpack_pairs(m["out"].astype(np.float32)) for m in res.results],
        axis=0)
    return out, res


def kernel(x, h, weight, bias):
    x = np.asarray(x, dtype=np.float32)
    h = np.asarray(h, dtype=np.float32)
    weight = np.asarray(weight, dtype=np.float32)
    bias = np.asarray(bias, dtype=np.float32)
    if np.any(bias != 0.0):
        # setup_inputs() always passes zero bias; keep a correct fallback.
        return _np_reference(x, h, weight, bias)
    out, _ = _run(x, h, weight, bias)
    return out
